# revision 39
# baseline (speedup 1.0000x reference)
"""Trainium2 Bass kernel for nn_BlockNet_89713276879602.

Sharding: data-parallel over batch B=8 across 8 cores (one batch element
per core, all weights replicated, no collectives).

Per-core layout: channels on partitions, sequence on the free dim.
- LayerNorm-over-C stats via ones-matmul on PE; stat math done packed
  (128,16) via a DRAM roundtrip; per-position R / mu*R broadcast across
  partitions via DRAM-source broadcast DMAs.
- Depthwise convs: q/ff1 as 5-tap scalar_tensor_tensor chains on DVE
  (bf16, with a shifted copy so odd taps stay 4B-aligned); kv (stride 2)
  and half of ff2 as diagonal-matmul accumulation on PE.
- Attention: logits computed transposed (key-position on partitions) so
  softmax exp is a single ACT pass per chunk; exp-sum folded into the
  attention-value matmul via an appended ones-row; 1/Z applied once on
  the concatenated per-head outputs before the output projection.
- BN/LN affine params and biases folded into conv weights on the host.

All matmuls run in bf16 with fp32 PSUM accumulation; the residual spine
(x2, y) stays fp32.
"""

import math

import numpy as np
import ml_dtypes

T, C, S, KT, F, H = 2, 256, 1024, 5, 1024, 8
HD = C // H
PB = 128
NCB = C // PB          # 2
NFB = F // PB          # 8
SK = S // 2            # 512
LQ = T * S + 1         # 2049
LQP = LQ + 3           # 2052 padded width
EPS = 1e-5
RSQ = float(1.0 / math.sqrt(HD))
N_CORES = 8

BF = ml_dtypes.bfloat16

_PROGRAM = None


def _build_program():
    import concourse.bass as bass
    import concourse.bacc as bacc
    import concourse.tile as tile
    from concourse import mybir
    from concourse.masks import make_identity

    f32 = mybir.dt.float32
    bf16 = mybir.dt.bfloat16
    AF = mybir.ActivationFunctionType
    ALU = mybir.AluOpType

    nc = bacc.Bacc("TRN2", enable_partition_id=False)

    # ---------------- DRAM I/O ----------------
    din = {}

    def dinp(name, shape, dt):
        din[name] = nc.dram_tensor(name, list(shape), dt, kind="ExternalInput")
        return din[name]

    x_bf = dinp("x_bf", (T, C, S), bf16)
    qpwT = dinp("qpwT", (T, C, C), bf16)
    kvpwT = dinp("kvpwT", (T, C, 2 * C), bf16)
    ff1pwT = dinp("ff1pwT", (T, C, F), bf16)
    ff2pwT = dinp("ff2pwT", (T, F, C), bf16)
    woutT = dinp("woutT", (C, C), bf16)
    wqclsT = dinp("wqclsT", (C, C), bf16)
    wff1T = dinp("wff1T", (C, F), bf16)
    wff2T = dinp("wff2T", (F, C), bf16)

    dwq = dinp("dwq", (T, C, KT), f32)
    dwkv = dinp("dwkv", (T, C, KT), f32)
    dwf1 = dinp("dwf1", (T, C, KT), f32)
    dwf2 = dinp("dwf2", (T, F, KT), f32)
    bdwq = dinp("bdwq", (T, C), f32)
    bdwkv = dinp("bdwkv", (T, C), f32)
    bdwf1 = dinp("bdwf1", (T, C), f32)
    bdwf2 = dinp("bdwf2", (T, F), f32)
    bpwq = dinp("bpwq", (T, C), f32)
    bpwkv = dinp("bpwkv", (T, 2 * C), f32)
    bpwf1 = dinp("bpwf1", (T, F), f32)
    bpwf2 = dinp("bpwf2", (T, C), f32)
    g1 = dinp("g1", (T, C), f32)
    b1 = dinp("b1", (T, C), f32)
    g2 = dinp("g2", (T, C), f32)
    b2 = dinp("b2", (T, C), f32)
    boutv = dinp("boutv", (C,), f32)
    bqclsv = dinp("bqclsv", (C,), f32)
    g1cv = dinp("g1cv", (C,), f32)
    b1cv = dinp("b1cv", (C,), f32)
    bff1f = dinp("bff1f", (F,), f32)
    bff2v = dinp("bff2v", (C,), f32)
    clsv = dinp("clsv", (C,), f32)

    y_out = nc.dram_tensor("y", [T, C, S], f32, kind="ExternalOutput")
    ycls_out = nc.dram_tensor("ycls", [C], f32, kind="ExternalOutput")

    def bcast_ap(dram_ap, npart, width):
        return bass.AP(tensor=dram_ap.tensor, offset=dram_ap.offset,
                       ap=[[0, npart], [1, width]])

    with tile.TileContext(nc) as tc:
        import contextlib
        est = contextlib.ExitStack()
        with est:
            wts = est.enter_context(tc.tile_pool(name="wts", bufs=1))
            keep = est.enter_context(tc.tile_pool(name="keep", bufs=1))
            dscr = est.enter_context(tc.tile_pool(name="dscr", bufs=1, space="DRAM"))
            work = est.enter_context(tc.tile_pool(name="work", bufs=2))
            stats = est.enter_context(tc.tile_pool(name="stats", bufs=2))
            w1pool = est.enter_context(tc.tile_pool(name="w1pool", bufs=2))
            w3pool = est.enter_context(tc.tile_pool(name="w3pool", bufs=2))
            ffw = est.enter_context(tc.tile_pool(name="ffw", bufs=3))
            epool = est.enter_context(tc.tile_pool(name="epool", bufs=8))
            a2pool = est.enter_context(tc.tile_pool(name="a2pool", bufs=2))
            # PSUM budget (8 banks total): pst 1 + pbig 2 + pdw 1 + plog 3 + pav 1
            pst = est.enter_context(tc.tile_pool(name="pst", bufs=1, space="PSUM"))
            pbig = est.enter_context(tc.tile_pool(name="pbig", bufs=2, space="PSUM"))
            pdw = est.enter_context(tc.tile_pool(name="pdw", bufs=1, space="PSUM"))
            plog = est.enter_context(tc.tile_pool(name="plog", bufs=1, space="PSUM"))
            pav = est.enter_context(tc.tile_pool(name="pav", bufs=1, space="PSUM"))

            # ------------- resident weights -------------
            qpw_s = wts.tile([PB, T, NCB, C], bf16)
            nc.sync.dma_start(out=qpw_s, in_=qpwT.ap().rearrange(
                "t (kc p) co -> p t kc co", p=PB))
            kvpw_s = wts.tile([PB, T, NCB, 2 * C], bf16)
            nc.sync.dma_start(out=kvpw_s, in_=kvpwT.ap().rearrange(
                "t (kc p) co -> p t kc co", p=PB))
            wout_s = wts.tile([PB, NCB, C], bf16)
            nc.sync.dma_start(out=wout_s, in_=woutT.ap().rearrange(
                "(kc p) co -> p kc co", p=PB))
            wqcls_s = wts.tile([PB, NCB, C], bf16)
            nc.sync.dma_start(out=wqcls_s, in_=wqclsT.ap().rearrange(
                "(kc p) co -> p kc co", p=PB))
            wff1_s = wts.tile([PB, NCB, F], bf16)
            nc.sync.dma_start(out=wff1_s, in_=wff1T.ap().rearrange(
                "(kc p) co -> p kc co", p=PB))
            wff2_s = wts.tile([PB, NFB, C], bf16)
            nc.sync.dma_start(out=wff2_s, in_=wff2T.ap().rearrange(
                "(kf p) co -> p kf co", p=PB))

            def load_tc(dram, nblk, nm):  # (T, nblk*128) -> (128, T, nblk)
                t_ = wts.tile([PB, T, nblk], f32, tag=nm, name=nm)
                nc.sync.dma_start(out=t_, in_=dram.ap().rearrange(
                    "t (cb p) -> p t cb", p=PB))
                return t_

            def load_c(dram, nblk, nm):   # (nblk*128,) -> (128, nblk)
                t_ = wts.tile([PB, nblk], f32, tag=nm, name=nm)
                nc.sync.dma_start(out=t_, in_=dram.ap().rearrange(
                    "(cb p) -> p cb", p=PB))
                return t_

            dwq_s = wts.tile([PB, T, NCB, KT], f32)
            nc.sync.dma_start(out=dwq_s, in_=dwq.ap().rearrange(
                "t (cb p) k -> p t cb k", p=PB))
            dwkv_s = wts.tile([PB, T, NCB, KT], f32)
            nc.sync.dma_start(out=dwkv_s, in_=dwkv.ap().rearrange(
                "t (cb p) k -> p t cb k", p=PB))
            dwf1_s = wts.tile([PB, T, NCB, KT], f32)
            nc.sync.dma_start(out=dwf1_s, in_=dwf1.ap().rearrange(
                "t (cb p) k -> p t cb k", p=PB))
            dwf2_s = wts.tile([PB, T, NFB, KT], f32)
            nc.sync.dma_start(out=dwf2_s, in_=dwf2.ap().rearrange(
                "t (fb p) k -> p t fb k", p=PB))

            bdwq_s = load_tc(bdwq, NCB, "bdwq_s")
            bdwkv_s = load_tc(bdwkv, NCB, "bdwkv_s")
            bdwf1_s = load_tc(bdwf1, NCB, "bdwf1_s")
            bdwf2_s = load_tc(bdwf2, NFB, "bdwf2_s")
            bpwq_s = load_tc(bpwq, NCB, "bpwq_s")
            bpwkv_s = load_tc(bpwkv, 2 * NCB, "bpwkv_s")
            bpwf1_s = load_tc(bpwf1, NFB, "bpwf1_s")
            bpwf2_s = load_tc(bpwf2, NCB, "bpwf2_s")
            g1_s = load_tc(g1, NCB, "g1_s")
            b1_s = load_tc(b1, NCB, "b1_s")
            g2_s = load_tc(g2, NCB, "g2_s")
            b2_s = load_tc(b2, NCB, "b2_s")
            bout_s = load_c(boutv, NCB, "bout_s")
            bqcls_s = load_c(bqclsv, NCB, "bqcls_s")
            bff1_s = load_c(bff1f, NFB, "bff1_s")
            bff2_s = load_c(bff2v, NCB, "bff2_s")

            g1c_row = wts.tile([1, C], f32)
            nc.sync.dma_start(out=g1c_row, in_=g1cv.ap().rearrange("(o c) -> o c", o=1))
            b1c_row = wts.tile([1, C], f32)
            nc.sync.dma_start(out=b1c_row, in_=b1cv.ap().rearrange("(o c) -> o c", o=1))
            cls_row = wts.tile([1, C], f32)
            nc.sync.dma_start(out=cls_row, in_=clsv.ap().rearrange("(o c) -> o c", o=1))

            ones_bf = wts.tile([PB, 1], bf16)
            nc.vector.memset(ones_bf, 1.0)
            eps_col = wts.tile([PB, 1], f32)
            nc.vector.memset(eps_col, EPS)
            eps_1 = wts.tile([1, 1], f32)
            nc.vector.memset(eps_1, EPS)
            ident = wts.tile([PB, PB], bf16)
            make_identity(nc, ident)

            # v bias broadcast rows (per t): (128, 256) from bpwkv[t, 256:512]
            bvbc = []
            for t in range(T):
                bt = wts.tile([PB, C], f32, tag=f"bvbc_{t}", name=f"bvbc_{t}")
                nc.sync.dma_start(out=bt, in_=bcast_ap(bpwkv.ap()[t, C:2 * C], PB, C))
                bvbc.append(bt)

            # --- touch pass: advance each engine's clock past startup loads
            # (walrus allows only one sync-wait on several ISA structs, so
            # compute ops must not need waits on weight DMAs + gpsimd)
            tch_v = wts.tile([1, 64], f32, tag="tch_v", name="tch_v")
            tch_a = wts.tile([1, 64], f32, tag="tch_a", name="tch_a")
            dve_touch = [qpw_s, kvpw_s, wout_s, wqcls_s,
                         wff1_s, wff2_s, dwq_s, dwkv_s, dwf1_s, dwf2_s,
                         bdwq_s, bdwkv_s, bdwf1_s, bdwf2_s, bpwq_s, bpwkv_s,
                         bpwf1_s, bpwf2_s, g1_s, b1_s, g2_s, b2_s, bout_s,
                         bqcls_s, bff1_s, bff2_s, g1c_row, b1c_row, cls_row,
                         ident] + bvbc
            for i, ap_ in enumerate(dve_touch):
                sl = ap_
                while len(sl.shape) > 2:
                    sl = sl[:, 0]
                nc.vector.tensor_copy(tch_v[:, i:i + 1], sl[0:1, 0:1])
            act_touch = [bpwq_s, bpwkv_s, bpwf1_s, bqcls_s, bff1_s, dwq_s]
            for i, ap_ in enumerate(act_touch):
                sl = ap_
                while len(sl.shape) > 2:
                    sl = sl[:, 0]
                nc.scalar.copy(out=tch_a[:, i:i + 1], in_=sl[0:1, 0:1])
            pe_touch = [qpw_s[:, 0, 0, 0:128], kvpw_s[:, 0, 0, 0:128],
                        wout_s[:, 0, 0:128], wqcls_s[:, 0, 0:128],
                        wff1_s[:, 0, 0:128], wff2_s[:, 0, 0:128],
                        ident[:, 0:128]]
            for ap_ in pe_touch:
                nc.tensor.ldweights(ap_)

            # ---------------- helpers ----------------
            def ln_stats(src_tiles, sq_tiles, pst, tag):
                """Returns (Rbc, MRbc) bf16 (128, S) broadcast tiles."""
                scrow = dscr.tile([2 * S], f32, tag=f"scrow_{tag}")
                scrow2 = scrow.rearrange("(h ns s) -> h ns s", h=2, ns=2)
                for half, tiles in ((0, src_tiles), (1, sq_tiles)):
                    for ns in range(2):
                        ps = pst.tile([1, 512], f32, tag="pst")
                        for cb in range(NCB):
                            nc.tensor.matmul(
                                ps, ones_bf, tiles[cb][:, ns * 512:(ns + 1) * 512],
                                start=(cb == 0), stop=(cb == NCB - 1))
                        srow = stats.tile([1, 512], f32, tag="srow")
                        nc.scalar.copy(out=srow, in_=ps)
                        nc.sync.dma_start(
                            out=scrow2[half, ns:ns + 1, :], in_=srow)
                stpk = stats.tile([PB, 16], f32, tag="stpk")
                nc.sync.dma_start(out=stpk, in_=scrow.rearrange("(i p) -> p i", p=PB))
                mu = stats.tile([PB, 8], f32, tag="mu")
                nc.vector.tensor_scalar(out=mu, in0=stpk[:, 0:8], scalar1=1.0 / C,
                                        scalar2=None, op0=ALU.mult)
                ex2 = stats.tile([PB, 8], f32, tag="ex2")
                nc.vector.tensor_scalar(out=ex2, in0=stpk[:, 8:16], scalar1=1.0 / C,
                                        scalar2=None, op0=ALU.mult)
                var = stats.tile([PB, 8], f32, tag="var")
                nc.vector.tensor_mul(var, mu, mu)
                nc.vector.tensor_sub(var, ex2, var)
                sd = stats.tile([PB, 8], f32, tag="sd")
                nc.scalar.activation(out=sd, in_=var, func=AF.Sqrt, bias=eps_col,
                                     scale=1.0)
                rr = stats.tile([PB, 8], f32, tag="rr")
                nc.vector.reciprocal_approx_fast(out=rr, in_=sd)
                mr = stats.tile([PB, 8], f32, tag="mr")
                nc.vector.tensor_mul(mr, mu, rr)
                rrb = stats.tile([PB, 8], bf16, tag="rrb")
                nc.vector.tensor_copy(rrb, rr)
                mrb = stats.tile([PB, 8], bf16, tag="mrb")
                nc.vector.tensor_copy(mrb, mr)
                scr = dscr.tile([S], bf16, tag=f"scR_{tag}")
                nc.sync.dma_start(out=scr.rearrange("(i p) -> p i", p=PB), in_=rrb)
                scmr = dscr.tile([S], bf16, tag=f"scMR_{tag}")
                nc.sync.dma_start(out=scmr.rearrange("(i p) -> p i", p=PB), in_=mrb)
                rbc = work.tile([PB, S], bf16, tag="rbc")
                nc.sync.dma_start(out=rbc, in_=bcast_ap(scr, PB, S))
                mrbc = work.tile([PB, S], bf16, tag="mrbc")
                nc.sync.dma_start(out=mrbc, in_=bcast_ap(scmr, PB, S))
                return rbc, mrbc

            def ln_apply(xb_cb, rbc, mrbc, g_sl, b_sl, xnp0_view):
                mrg = work.tile([PB, S], bf16, tag="mrg")
                nc.vector.tensor_scalar(out=mrg, in0=mrbc, scalar1=g_sl, scalar2=b_sl,
                                        op0=ALU.mult, op1=ALU.subtract)
                w1 = work.tile([PB, S], bf16, tag="w1")
                nc.vector.tensor_mul(w1, xb_cb, rbc)
                nc.vector.scalar_tensor_tensor(out=xnp0_view, in0=w1, scalar=g_sl,
                                               in1=mrg, op0=ALU.mult, op1=ALU.subtract)

            def pad_tile(pool, tag):
                p0 = pool.tile([PB, S + 4], bf16, tag=tag, name=tag)
                nc.vector.memset(p0[:, 0:2], 0.0)
                nc.vector.memset(p0[:, S + 2:S + 4], 0.0)
                return p0

            def dw_dve(p0, w_sl, bias_sl, pool, tag, out=None):
                # 5-tap depthwise + bias + relu; returns the result tile
                acc = pool.tile([PB, S], bf16, tag=tag)
                nc.vector.tensor_scalar(out=acc, in0=p0[:, 0:S], scalar1=w_sl[0],
                                        scalar2=None, op0=ALU.mult)
                for j in range(1, KT):
                    nc.vector.scalar_tensor_tensor(
                        out=acc, in0=p0[:, j:j + S], scalar=w_sl[j], in1=acc,
                        op0=ALU.mult, op1=ALU.add)
                dst = acc if out is None else out
                nc.vector.tensor_scalar(out=dst, in0=acc, scalar1=bias_sl,
                                        scalar2=0.0, op0=ALU.add, op1=ALU.max)
                return dst

            def dw_pe(psum, p0, w_sl, stride, ncols, col0, dpool):
                # accumulate 5 diag matmuls into psum (128, ncols)
                for j in range(KT):
                    dg = dpool.tile([PB, PB], bf16, tag="diag")
                    nc.vector.tensor_scalar(out=dg, in0=ident, scalar1=w_sl[j],
                                            scalar2=None, op0=ALU.mult)
                    rhs = p0[:, j + col0 * stride: j + col0 * stride + ncols * stride: stride] \
                        if stride > 1 else p0[:, j + col0: j + col0 + ncols]
                    nc.tensor.matmul(psum, dg, rhs, start=(j == 0), stop=(j == KT - 1))

            # ================= PHASE 1: LN1 + q/kv convs + cls =================
            xnp0 = {}
            q_s = {}
            k_s = {}
            vT = {}
            if True:
                pst1, pbig1, pdw1 = pst, pbig, pdw
                for t in range(T):
                    xb = []
                    sq = []
                    for cb in range(NCB):
                        xt = w1pool.tile([PB, S], bf16, tag="xb")
                        nc.sync.dma_start(
                            out=xt, in_=x_bf.ap()[t, cb * PB:(cb + 1) * PB, :])
                        xb.append(xt)
                        st = w1pool.tile([PB, S], bf16, tag="sq")
                        nc.vector.tensor_mul(st, xt, xt)
                        sq.append(st)
                    rbc, mrbc = ln_stats(xb, sq, pst1, f"ln1_{t}")
                    for cb in range(NCB):
                        p0 = pad_tile(keep, f"xnp0_{t}{cb}")
                        ln_apply(xb[cb], rbc, mrbc, g1_s[:, t, cb:cb + 1],
                                 b1_s[:, t, cb:cb + 1], p0[:, 2:2 + S])
                        xnp0[(t, cb)] = p0

                    # --- q dw (DVE) + pw ---
                    rq = []
                    for cb in range(NCB):
                        wsl = [dwq_s[:, t, cb, j:j + 1] for j in range(KT)]
                        r = dw_dve(xnp0[(t, cb)], wsl, bdwq_s[:, t, cb:cb + 1],
                                   w1pool, "qacc")
                        rq.append(r)
                    for mc in range(NCB):
                        qs = keep.tile([PB, S], bf16, tag=f"q_{t}{mc}")
                        for ns in range(2):
                            ps = pbig1.tile([PB, 512], f32, tag="pbig")
                            for kc in range(NCB):
                                nc.tensor.matmul(
                                    ps, qpw_s[:, t, kc, mc * PB:(mc + 1) * PB],
                                    rq[kc][:, ns * 512:(ns + 1) * 512],
                                    start=(kc == 0), stop=(kc == NCB - 1))
                            nc.scalar.activation(
                                out=qs[:, ns * 512:(ns + 1) * 512], in_=ps,
                                func=AF.Identity, bias=bpwq_s[:, t, mc:mc + 1], scale=1.0)
                        q_s[(t, mc)] = qs

                    # --- kv dw (PE diag, stride 2) + pw ---
                    rkv = []
                    for cb in range(NCB):
                        ps = pdw1.tile([PB, 512], f32, tag="pdw")
                        wsl = [dwkv_s[:, t, cb, j:j + 1] for j in range(KT)]
                        dw_pe(ps, xnp0[(t, cb)], wsl, 2, SK, 0, w1pool)
                        r = w1pool.tile([PB, SK], bf16, tag="rkv")
                        nc.vector.tensor_scalar(out=r, in0=ps,
                                                scalar1=bdwkv_s[:, t, cb:cb + 1],
                                                scalar2=0.0, op0=ALU.add, op1=ALU.max)
                        rkv.append(r)
                    for mc in range(NCB):  # k part: (c,s) layout
                        ks = keep.tile([PB, SK], bf16, tag=f"k_{t}{mc}")
                        ps = pbig1.tile([PB, 512], f32, tag="pbig")
                        for kc in range(NCB):
                            nc.tensor.matmul(ps, kvpw_s[:, t, kc, mc * PB:(mc + 1) * PB],
                                             rkv[kc], start=(kc == 0), stop=(kc == NCB - 1))
                        nc.scalar.activation(out=ks, in_=ps, func=AF.Identity,
                                             bias=bpwkv_s[:, t, mc:mc + 1], scale=1.0)
                        k_s[(t, mc)] = ks
                    for sm in range(4):  # v part: (kpos, c) layout
                        m = t * 4 + sm
                        vt = keep.tile([PB, H, HD + 1], bf16, tag=f"vT_{m}")
                        ps = pbig1.tile([PB, 512], f32, tag="pbig")
                        for kc in range(NCB):
                            nc.tensor.matmul(
                                ps[:, 0:C], rkv[kc][:, sm * PB:(sm + 1) * PB],
                                kvpw_s[:, t, kc, C:2 * C],
                                start=(kc == 0), stop=(kc == NCB - 1))
                        nc.vector.tensor_add(
                            vt[:, :, 0:HD],
                            ps[:, 0:C].rearrange("p (h d) -> p h d", h=H),
                            bvbc[t].rearrange("p (h d) -> p h d", h=H))
                        nc.vector.memset(vt[:, :, HD:HD + 1], 1.0)
                        vT[m] = vt

                # --- cls prep ---
                st6 = stats.tile([1, 6], f32, tag="st6")
                nc.vector.bn_stats(out=st6, in_=cls_row)
                mv = stats.tile([1, 2], f32, tag="mv")
                nc.vector.bn_aggr(out=mv, in_=st6)
                sd1 = stats.tile([1, 1], f32, tag="sd1")
                nc.scalar.activation(out=sd1, in_=mv[:, 1:2], func=AF.Sqrt,
                                     bias=eps_1, scale=1.0)
                rc1 = stats.tile([1, 1], f32, tag="rc1")
                nc.vector.reciprocal_approx_fast(out=rc1, in_=sd1)
                ucl = stats.tile([1, C], f32, tag="ucl", bufs=1)
                nc.vector.tensor_scalar(out=ucl, in0=cls_row, scalar1=mv[:, 0:1],
                                        scalar2=rc1, op0=ALU.subtract, op1=ALU.mult)
                clsn_row = keep.tile([1, C], f32, tag="clsn_row")
                nc.vector.tensor_mul(clsn_row, ucl, g1c_row)
                nc.vector.tensor_add(clsn_row, clsn_row, b1c_row)
                sccls = dscr.tile([C], f32, tag="sccls")
                nc.sync.dma_start(out=sccls.rearrange("(o c) -> o c", o=1),
                                  in_=clsn_row)
                clsC = keep.tile([PB, NCB], f32, tag="clsC")
                nc.sync.dma_start(out=clsC, in_=sccls.rearrange("(cb p) -> p cb", p=PB))
                clsCb = keep.tile([PB, NCB], bf16, tag="clsCb")
                nc.vector.tensor_copy(clsCb, clsC)
                qcls_s = keep.tile([PB, NCB], bf16, tag="qcls")
                for mc in range(NCB):
                    psq = pbig1.tile([PB, 512], f32, tag="pbig")
                    for kc in range(NCB):
                        nc.tensor.matmul(psq[:, 0:1],
                                         wqcls_s[:, kc, mc * PB:(mc + 1) * PB],
                                         clsCb[:, kc:kc + 1],
                                         start=(kc == 0), stop=(kc == NCB - 1))
                    nc.scalar.activation(out=qcls_s[:, mc:mc + 1], in_=psq[:, 0:1],
                                         func=AF.Identity,
                                         bias=bqcls_s[:, mc:mc + 1], scale=1.0)

            # ================= PHASE 2: attention =================
            avC = [keep.tile([PB, LQP], bf16, tag=f"avC_{cb}", name=f"avC_{cb}")
                   for cb in range(NCB)]
            zbc = [keep.tile([PB, LQP], bf16, tag=f"zbc_{cb}", name=f"zbc_{cb}")
                   for cb in range(NCB)]
            avn = avC
            if True:
                for h in range(H):
                    cb = h // 4
                    po = 32 * (h % 4)
                    etiles = []
                    for m in range(8):
                        tk, sm = m // 4, m % 4
                        lhs_k = k_s[(tk, cb)][po:po + 32, sm * PB:(sm + 1) * PB]
                        em = epool.tile([PB, LQP], bf16, tag="E")
                        pl = plog.tile([PB, 1026], f32, tag="plog")
                        for ns in range(2):
                            nc.tensor.matmul(pl[:, ns * 512:(ns + 1) * 512], lhs_k,
                                             q_s[(0, cb)][po:po + 32, ns * 512:(ns + 1) * 512],
                                             start=True, stop=True, tile_position=(po, 0))
                        nc.scalar.activation(out=em[:, 0:S], in_=pl[:, 0:S],
                                             func=AF.Exp, bias=0.0, scale=RSQ)
                        pl2 = plog.tile([PB, 1026], f32, tag="plog")
                        for ns in range(2):
                            nc.tensor.matmul(pl2[:, ns * 512:(ns + 1) * 512], lhs_k,
                                             q_s[(1, cb)][po:po + 32, ns * 512:(ns + 1) * 512],
                                             start=True, stop=True, tile_position=(po, 0))
                        nc.tensor.matmul(pl2[:, 1024:1025], lhs_k,
                                         qcls_s[po:po + 32, cb:cb + 1],
                                         start=True, stop=True, tile_position=(po, 0))
                        nc.scalar.activation(out=em[:, S:S + 1025], in_=pl2[:, 0:1025],
                                             func=AF.Exp, bias=0.0, scale=RSQ)
                        etiles.append(em)
                    av33 = a2pool.tile([HD + 1, LQP], bf16, tag="av33", bufs=1)
                    nc.vector.memset(av33[:, LQ:LQP], 1.0)
                    for g in range(5):
                        ng = 512 if g < 4 else 1
                        pv = pav.tile([HD + 1, 512], f32, tag="pav")
                        for m in range(8):
                            nc.tensor.matmul(pv[:, 0:ng], vT[m][:, h, :],
                                             etiles[m][:, g * 512:g * 512 + ng],
                                             start=(m == 0), stop=(m == 7))
                        nc.vector.tensor_copy(av33[:, g * 512:g * 512 + ng], pv[:, 0:ng])
                    nc.sync.dma_start(out=avC[cb][po:po + 32, :], in_=av33[0:32, :])
                    scz = dscr.tile([LQP], bf16, tag="scz")
                    nc.sync.dma_start(out=scz.rearrange("(o q) -> o q", o=1),
                                      in_=av33[32:33, :])
                    nc.sync.dma_start(out=zbc[cb][po:po + 32, :],
                                      in_=bcast_ap(scz, 32, LQP))
                for cb in range(NCB):
                    zf = a2pool.tile([PB, LQP], f32, tag="zf", bufs=1)
                    nc.vector.tensor_copy(zf, zbc[cb])
                    nc.vector.reciprocal_approx_fast(out=zf, in_=zf)
                    nc.vector.tensor_mul(avC[cb], avC[cb], zf)

            # ============ PHASE 3a: wout + x2 + xc + LN2 ============
            x2 = {}
            x2b = {}
            xn2p0 = {}
            xc = keep.tile([PB, NCB], f32, tag="xc")
            if True:
                pbig3, pst3 = pbig, pst
                for tq in range(T):
                    for mc in range(NCB):
                        # shares the q slot (q is dead after the logits matmuls)
                        x2[(tq, mc)] = keep.tile([PB, S], f32, tag=f"q_{tq}{mc}",
                                                 name=f"x2_{tq}{mc}")
                for mc in range(NCB):
                    for g in range(5):
                        ng = 512 if g < 4 else 1
                        ps = pbig3.tile([PB, 512], f32, tag="pbig")
                        for kc in range(NCB):
                            nc.tensor.matmul(ps[:, 0:ng],
                                             wout_s[:, kc, mc * PB:(mc + 1) * PB],
                                             avn[kc][:, g * 512:g * 512 + ng],
                                             start=(kc == 0), stop=(kc == NCB - 1))
                        if g < 4:
                            tq, ns = g // 2, g % 2
                            nc.vector.scalar_tensor_tensor(
                                out=x2[(tq, mc)][:, ns * 512:(ns + 1) * 512],
                                in0=ps[:, 0:512], scalar=bout_s[:, mc:mc + 1],
                                in1=xnp0[(tq, mc)][:, 2 + ns * 512:2 + (ns + 1) * 512],
                                op0=ALU.add, op1=ALU.add)
                        else:
                            nc.vector.scalar_tensor_tensor(
                                out=xc[:, mc:mc + 1], in0=ps[:, 0:1],
                                scalar=bout_s[:, mc:mc + 1], in1=clsC[:, mc:mc + 1],
                                op0=ALU.add, op1=ALU.add)
                for t in range(T):
                    xb2 = []
                    sq2 = []
                    for cb in range(NCB):
                        xb_t = w3pool.tile([PB, S], bf16, tag="x2b")
                        nc.vector.tensor_copy(xb_t, x2[(t, cb)])
                        x2b[(t, cb)] = xb_t
                        st = w3pool.tile([PB, S], bf16, tag="sq2")
                        nc.vector.tensor_mul(st, xb_t, xb_t)
                        xb2.append(xb_t)
                        sq2.append(st)
                    rbc, mrbc = ln_stats(xb2, sq2, pst3, f"ln2_{t}")
                    for cb in range(NCB):
                        # shares the xnp0 slot (xnp0 is dead after x2)
                        p0 = pad_tile(keep, f"xnp0_{t}{cb}")
                        ln_apply(xb2[cb], rbc, mrbc, g2_s[:, t, cb:cb + 1],
                                 b2_s[:, t, cb:cb + 1], p0[:, 2:2 + S])
                        xn2p0[(t, cb)] = p0

                    # FFN for this t
                    if True:
                        pdw3 = pdw
                        ff1w = ffw.tile([PB, NCB, F], bf16, tag="ff1w", bufs=1)
                        nc.sync.dma_start(out=ff1w, in_=ff1pwT.ap()[t].rearrange(
                            "(kc p) co -> p kc co", p=PB))
                        ff2w = ffw.tile([PB, NFB, C], bf16, tag="ff2w", bufs=1)
                        nc.sync.dma_start(out=ff2w, in_=ff2pwT.ap()[t].rearrange(
                            "(kf p) co -> p kf co", p=PB))
                        nc.tensor.ldweights(ff1w[:, 0, 0:128])
                        nc.tensor.ldweights(ff2w[:, 0, 0:128])
                        rf1 = []
                        for cb in range(NCB):
                            wsl = [dwf1_s[:, t, cb, j:j + 1] for j in range(KT)]
                            r = dw_dve(xn2p0[(t, cb)], wsl, bdwf1_s[:, t, cb:cb + 1],
                                       w3pool, "f1acc")
                            rf1.append(r)
                        rf2_all = []
                        for fm in range(NFB):
                            hp0 = epool.tile([PB, S + 4], bf16, tag="E")
                            nc.vector.memset(hp0[:, 0:2], 0.0)
                            nc.vector.memset(hp0[:, S + 2:S + 4], 0.0)
                            for ns in range(2):
                                ps = pbig3.tile([PB, 512], f32, tag="pbig")
                                for kc in range(NCB):
                                    nc.tensor.matmul(
                                        ps, ff1w[:, kc, fm * PB:(fm + 1) * PB],
                                        rf1[kc][:, ns * 512:(ns + 1) * 512],
                                        start=(kc == 0), stop=(kc == NCB - 1))
                                nc.scalar.activation(
                                    out=hp0[:, 2 + ns * 512:2 + (ns + 1) * 512],
                                    in_=ps, func=AF.Relu,
                                    bias=bpwf1_s[:, t, fm:fm + 1], scale=1.0)
                            wsl = [dwf2_s[:, t, fm, j:j + 1] for j in range(KT)]
                            rf2 = ffw.tile([PB, S], bf16, tag="rf2", bufs=9)
                            if fm < 4:  # PE diag path
                                for ns in range(2):
                                    psd = pdw3.tile([PB, 512], f32, tag="pdw")
                                    dw_pe(psd, hp0, wsl, 1, 512, ns * 512, w3pool)
                                    nc.vector.tensor_scalar(
                                        out=rf2[:, ns * 512:(ns + 1) * 512], in0=psd,
                                        scalar1=bdwf2_s[:, t, fm:fm + 1], scalar2=0.0,
                                        op0=ALU.add, op1=ALU.max)
                            else:       # DVE path (odd taps run 1x)
                                dw_dve(hp0, wsl, bdwf2_s[:, t, fm:fm + 1],
                                       w1pool, "qacc", out=rf2)
                            rf2_all.append(rf2)
                        for mc in range(NCB):
                            for ns in range(2):
                                ps = pbig3.tile([PB, 512], f32, tag="pbig")
                                for kf in range(NFB):
                                    nc.tensor.matmul(
                                        ps, ff2w[:, kf, mc * PB:(mc + 1) * PB],
                                        rf2_all[kf][:, ns * 512:(ns + 1) * 512],
                                        start=(kf == 0), stop=(kf == NFB - 1))
                                ys = epool.tile([PB, 512], f32, tag="E")
                                nc.vector.scalar_tensor_tensor(
                                    out=ys, in0=ps,
                                    scalar=bpwf2_s[:, t, mc:mc + 1],
                                    in1=x2[(t, mc)][:, ns * 512:(ns + 1) * 512],
                                    op0=ALU.add, op1=ALU.add)
                                nc.sync.dma_start(
                                    out=y_out.ap()[t, mc * PB:(mc + 1) * PB,
                                                   ns * 512:(ns + 1) * 512],
                                    in_=ys)

                # ---- cls MLP ----
                xcb = w3pool.tile([PB, NCB], bf16, tag="xcb")
                nc.vector.tensor_copy(xcb, xc)
                sqc = w3pool.tile([PB, NCB], bf16, tag="sqc")
                nc.vector.tensor_mul(sqc, xcb, xcb)
                psx = pst3.tile([1, 512], f32, tag="pst")
                for cb in range(NCB):
                    nc.tensor.matmul(psx[:, 0:1], ones_bf, xcb[:, cb:cb + 1],
                                     start=(cb == 0), stop=(cb == NCB - 1))
                for cb in range(NCB):
                    nc.tensor.matmul(psx[:, 1:2], ones_bf, sqc[:, cb:cb + 1],
                                     start=(cb == 0), stop=(cb == NCB - 1))
                ms = stats.tile([1, 2], f32, tag="ms")
                nc.scalar.copy(out=ms, in_=psx[:, 0:2])
                muc = stats.tile([1, 2], f32, tag="muc")  # [mu, ex2]
                nc.vector.tensor_scalar(out=muc, in0=ms, scalar1=1.0 / C,
                                        scalar2=None, op0=ALU.mult)
                varc = stats.tile([1, 1], f32, tag="varc")
                nc.vector.tensor_mul(varc, muc[:, 0:1], muc[:, 0:1])
                nc.vector.tensor_sub(varc, muc[:, 1:2], varc)
                sdc = stats.tile([1, 1], f32, tag="sdc")
                nc.scalar.activation(out=sdc, in_=varc, func=AF.Sqrt, bias=eps_1,
                                     scale=1.0)
                rcc = stats.tile([1, 1], f32, tag="rcc")
                nc.vector.reciprocal_approx_fast(out=rcc, in_=sdc)
                mrc = stats.tile([1, 2], f32, tag="mrc")  # [R, MR]
                nc.vector.tensor_copy(mrc[:, 0:1], rcc)
                nc.vector.tensor_mul(mrc[:, 1:2], muc[:, 0:1], rcc)
                scc2 = dscr.tile([2], f32, tag="scc2")
                nc.sync.dma_start(out=scc2.rearrange("(o c) -> o c", o=1), in_=mrc)
                rmr = w3pool.tile([PB, 2], f32, tag="rmr")
                nc.sync.dma_start(out=rmr, in_=bcast_ap(scc2, PB, 2))
                ucn = w3pool.tile([PB, NCB], bf16, tag="ucn")
                nc.vector.tensor_scalar(out=ucn, in0=xc, scalar1=rmr[:, 0:1],
                                        scalar2=rmr[:, 1:2], op0=ALU.mult,
                                        op1=ALU.subtract)
                h1c = w3pool.tile([PB, NFB], bf16, tag="h1c")
                for fm in range(NFB):
                    ps = pbig3.tile([PB, 512], f32, tag="pbig")
                    for kc in range(NCB):
                        nc.tensor.matmul(ps[:, 0:1],
                                         wff1_s[:, kc, fm * PB:(fm + 1) * PB],
                                         ucn[:, kc:kc + 1],
                                         start=(kc == 0), stop=(kc == NCB - 1))
                    nc.scalar.activation(out=h1c[:, fm:fm + 1], in_=ps[:, 0:1],
                                         func=AF.Relu, bias=bff1_s[:, fm:fm + 1],
                                         scale=1.0)
                ycl = w3pool.tile([PB, NCB], f32, tag="ycl")
                for mc in range(NCB):
                    ps = pbig3.tile([PB, 512], f32, tag="pbig")
                    for kf in range(NFB):
                        nc.tensor.matmul(ps[:, 0:1],
                                         wff2_s[:, kf, mc * PB:(mc + 1) * PB],
                                         h1c[:, kf:kf + 1],
                                         start=(kf == 0), stop=(kf == NFB - 1))
                    nc.vector.scalar_tensor_tensor(
                        out=ycl[:, mc:mc + 1], in0=ps[:, 0:1],
                        scalar=bff2_s[:, mc:mc + 1], in1=xc[:, mc:mc + 1],
                        op0=ALU.add, op1=ALU.add)
                nc.sync.dma_start(out=ycls_out.ap().rearrange("(cb p) -> p cb", p=PB),
                                  in_=ycl)

    nc.compile()
    return nc


def _get_program():
    global _PROGRAM
    if _PROGRAM is None:
        _PROGRAM = _build_program()
    return _PROGRAM


def _fold_sepconv(dw_w, dw_b, bn_s, bn_b):
    return dw_w * bn_s[:, None], bn_s * dw_b + bn_b


def make_core_inputs(b, w):
    """Host-side folds + per-core input map for batch element b."""
    f32 = np.float32

    def seq(*arrs):
        return np.stack(arrs, 0)

    dwq_l, bdwq_l, dwkv_l, bdwkv_l, dwf1_l, bdwf1_l, dwf2_l, bdwf2_l = \
        [], [], [], [], [], [], [], []
    for t in range(T):
        a, bb = _fold_sepconv(w["q_dw_w"][t], w["q_dw_b"][t], w["q_bn_s"][t], w["q_bn_b"][t])
        dwq_l.append(a); bdwq_l.append(bb)
        a, bb = _fold_sepconv(w["kv_dw_w"][t], w["kv_dw_b"][t], w["kv_bn_s"][t], w["kv_bn_b"][t])
        dwkv_l.append(a); bdwkv_l.append(bb)
        a, bb = _fold_sepconv(w["ff1_dw_w"][t], w["ff1_dw_b"][t], w["ff1_bn_s"][t], w["ff1_bn_b"][t])
        dwf1_l.append(a); bdwf1_l.append(bb)
        a, bb = _fold_sepconv(w["ff2_dw_w"][t], w["ff2_dw_b"][t], w["ff2_bn_s"][t], w["ff2_bn_b"][t])
        dwf2_l.append(a); bdwf2_l.append(bb)

    wff1_f = (w["wff1"] * w["ln2c_g"][None, :]).astype(f32)
    bff1_f = (w["bff1"] + w["wff1"] @ w["ln2c_b"]).astype(f32)

    m = {
        "x_bf": w["inp"][:, b].astype(BF),
        "qpwT": np.ascontiguousarray(w["q_pw_w"].transpose(0, 2, 1)).astype(BF),
        "kvpwT": np.ascontiguousarray(w["kv_pw_w"].transpose(0, 2, 1)).astype(BF),
        "ff1pwT": np.ascontiguousarray(w["ff1_pw_w"].transpose(0, 2, 1)).astype(BF),
        "ff2pwT": np.ascontiguousarray(w["ff2_pw_w"].transpose(0, 2, 1)).astype(BF),
        "woutT": np.ascontiguousarray(w["wout"].T).astype(BF),
        "wqclsT": np.ascontiguousarray(w["wq_cls"].T).astype(BF),
        "wff1T": np.ascontiguousarray(wff1_f.T).astype(BF),
        "wff2T": np.ascontiguousarray(w["wff2"].T).astype(BF),
        "dwq": seq(*dwq_l).astype(f32), "bdwq": seq(*bdwq_l).astype(f32),
        "dwkv": seq(*dwkv_l).astype(f32), "bdwkv": seq(*bdwkv_l).astype(f32),
        "dwf1": seq(*dwf1_l).astype(f32), "bdwf1": seq(*bdwf1_l).astype(f32),
        "dwf2": seq(*dwf2_l).astype(f32), "bdwf2": seq(*bdwf2_l).astype(f32),
        "bpwq": w["q_pw_b"].astype(f32), "bpwkv": w["kv_pw_b"].astype(f32),
        "bpwf1": w["ff1_pw_b"].astype(f32), "bpwf2": w["ff2_pw_b"].astype(f32),
        "g1": w["ln1_g"].astype(f32), "b1": w["ln1_b"].astype(f32),
        "g2": w["ln2_g"].astype(f32), "b2": w["ln2_b"].astype(f32),
        "boutv": w["bout"].astype(f32), "bqclsv": w["bq_cls"].astype(f32),
        "g1cv": w["ln1c_g"].astype(f32), "b1cv": w["ln1c_b"].astype(f32),
        "bff1f": bff1_f, "bff2v": w["bff2"].astype(f32),
        "clsv": w["cls"][b].astype(f32),
    }
    return m


def kernel(**inputs):
    import concourse.bass_utils as bass_utils

    w = {k: np.asarray(v, np.float32) for k, v in inputs.items()}
    B = w["inp"].shape[1]
    nc = _get_program()
    in_maps = [make_core_inputs(b, w) for b in range(B)]
    res = bass_utils.run_bass_kernel_spmd(nc, in_maps, core_ids=list(range(N_CORES)))
    y = np.stack([res.results[b]["y"] for b in range(B)], axis=1)
    y_cls = np.stack([res.results[b]["ycls"] for b in range(B)], axis=0)
    return y.astype(np.float32), y_cls.astype(np.float32)
